# revision 35
# baseline (speedup 1.0000x reference)
"""Trainium2 Bass kernel for a dense transformer block (pre-LN, 12-head attn + MLP).

Shapes (hardcoded): B=8, S=1024, D=768, H=12, DH=64, MLP=3072.
Query rows >= 512 have their attention scores zeroed pre-softmax, so their
context vector is mean(v) over all 1024 keys.

Sharding: data-parallel over batch - each of the 8 NeuronCores processes one
batch element; no collectives.

v2 design: fp8(e4m3) everywhere on the PE with DoubleRow (2 fp8 MACs/cell),
all weights SBUF-resident, LN gamma/beta folded into the weights on the host,
single ACT table set (ln/exp/relu/copy), feature-major MLP2 output with a
free host-side transpose.

Scale conventions (host pre-scales, on-chip rescales):
  weights fp8 = 64*W_eff, activations fp8 = 32*act. PSUM products carry
  2048x, rescaled during PSUM evacuation (1/64 into fp8 acts, 1/2048 into
  fp32). exp computed as exp(s_true - 2) for fp8 headroom (cancels in
  softmax).
"""
import sys

try:
    import concourse  # noqa: F401
except ImportError:
    sys.path.insert(0, "/opt/trn_rl_repo")

import numpy as np
from contextlib import ExitStack

import concourse.bass as bass  # noqa: F401
import concourse.tile as tile
from concourse import bacc, mybir
from concourse.bass import ts
from concourse.masks import make_identity

F32 = mybir.dt.float32
F32R = mybir.dt.float32r
F8 = mybir.dt.float8e4
BF16 = mybir.dt.bfloat16
I32 = mybir.dt.int32
AF = mybir.ActivationFunctionType
ALU = mybir.AluOpType
DR = mybir.MatmulPerfMode.DoubleRow

B, S, D = 8, 1024, 768
H, DH, MLP = 12, 64, 3072
SQ = 512          # live query rows (rows >= SQ get uniform attention)
EPS = 1e-6
KD = D // 128     # 6
NT = S // 128     # 8
MT = MLP // 128   # 24

# Schraudolph exp-via-int-bits: exp(sp/8192 - 2) ~= bitcast(int32(sp*SA + SB))
SA = float(2.0 ** 23 / (np.log(2.0) * 8192.0))
SB = float(127.0 * 2.0 ** 23 - 366393.0 - 2.0 * 2.0 ** 23 / np.log(2.0))


def build_program():
    nc = bacc.Bacc(
        "TRN2", target_bir_lowering=False, debug=False, enable_asserts=False
    )
    d = {}
    d["x"] = nc.dram_tensor("x", (S, D), F32, kind="ExternalInput").ap()
    for nm in ("wq", "wk", "wv", "wo"):
        d[nm] = nc.dram_tensor(nm, (128, 3, 2, D), F8, kind="ExternalInput").ap()
    d["w1b"] = nc.dram_tensor("w1b", (128, KD, MLP), BF16, kind="ExternalInput").ap()
    d["w2b"] = nc.dram_tensor("w2b", (128, MT, D), BF16, kind="ExternalInput").ap()
    for nm, n in (("bq32", D), ("bk32", D), ("b1s", MLP)):
        d[nm] = nc.dram_tensor(nm, (n,), F32, kind="ExternalInput").ap()
    for nm in ("bo_e", "b2e"):
        d[nm] = nc.dram_tensor(nm, (D,), F32R, kind="ExternalInput").ap()
    out_d = nc.dram_tensor("out", (D, S), BF16, kind="ExternalOutput").ap()

    with tile.TileContext(nc) as tc, ExitStack() as ctx:
        sg = ctx.enter_context(tc.tile_pool(name="sg", bufs=1))
        p_x = ctx.enter_context(tc.tile_pool(name="p_x", bufs=2))
        p_xn = ctx.enter_context(tc.tile_pool(name="p_xn", bufs=2))
        p_xb = ctx.enter_context(tc.tile_pool(name="p_xb", bufs=2))
        p_tiny = ctx.enter_context(tc.tile_pool(name="p_tiny", bufs=4))
        p_pbs = ctx.enter_context(tc.tile_pool(name="p_pbs", bufs=2))
        p_out = ctx.enter_context(tc.tile_pool(name="p_out", bufs=2))

        # ---- constants ----
        ident_bf = sg.tile([128, 128], BF16, name="ident_bf")
        make_identity(nc, ident_bf[:])
        eps_t = sg.tile([128, 1], F32, name="eps_t")
        nc.vector.memset(eps_t[:], EPS / 1024.0)
        negtwo = sg.tile([128, 1], F32, name="negtwo")
        nc.vector.memset(negtwo[:], -2.0)
        ones_r = sg.tile([1, 128], F32R, name="ones_r")
        nc.vector.memset(ones_r[:].bitcast(F32), 1.0)

        # ---- weights (SBUF-resident, fp8) ----
        wq_sb = sg.tile([128, 3, 2, D], F8, name="wq_sb")
        nc.sync.dma_start(wq_sb[:], d["wq"])
        wk_sb = sg.tile([128, 3, 2, D], F8, name="wk_sb")
        nc.sync.dma_start(wk_sb[:], d["wk"])
        wv_sb = sg.tile([128, 3, 2, D], F8, name="wv_sb")
        nc.sync.dma_start(wv_sb[:], d["wv"])
        wo_sb = sg.tile([128, 3, 2, D], F8, name="wo_sb")
        nc.sync.dma_start(wo_sb[:], d["wo"])
        p_w1 = ctx.enter_context(tc.tile_pool(name="p_w1", bufs=2))
        p_w2 = ctx.enter_context(tc.tile_pool(name="p_w2", bufs=2))

        # ---- biases ----
        bq_sb = sg.tile([128, KD], F32, name="bq_sb")
        nc.sync.dma_start(bq_sb[:], d["bq32"].rearrange("(t p) -> p t", p=128))
        bk_sb = sg.tile([128, KD], F32, name="bk_sb")
        nc.sync.dma_start(bk_sb[:], d["bk32"].rearrange("(t p) -> p t", p=128))
        b1_sb = sg.tile([128, MT], F32, name="b1_sb")
        nc.sync.dma_start(b1_sb[:], d["b1s"].rearrange("(t p) -> p t", p=128))

        with ExitStack() as sbc:
            ps_bc = sbc.enter_context(
                tc.tile_pool(name="ps_bc", bufs=2, space="PSUM"))

            def bcast_row(src_1d, name):
                row = p_tiny.tile([1, D], F32R, tag="row", name=f"row_{name}")
                nc.sync.dma_start(row[:], src_1d[None, :])
                t = sg.tile([128, D], F32, name=name)
                for half in range(2):
                    pbk = ps_bc.tile([128, 384], F32, tag="bc",
                                     name=f"bc_{name}{half}")
                    nc.tensor.matmul(pbk[:], ones_r[:], row[:, ts(half, 384)],
                                     start=True, stop=True)
                    nc.vector.tensor_copy(t[:, ts(half, 384)], pbk[:])
                return t

            bo_bc = bcast_row(d["bo_e"], "bo_bc")
            b2_bc = bcast_row(d["b2e"], "b2_bc")

        # ---- big state ----
        xnT = sg.tile([128, KD, S], F8, name="xnT")
        qt = sg.tile([128, KD, SQ], F8, name="qt")
        kt = sg.tile([128, KD, S], F8, name="kt")
        v_aug = sg.tile([128, NT, H, 68], F8, name="v_aug")
        nc.gpsimd.memset(v_aug[:, :, :, DH:DH + 1], 1.0)
        expT = [sg.tile([128, NT, 2, 264], F8, name=f"expT{i}") for i in range(2)]
        for i in range(2):
            nc.gpsimd.memset(expT[i][:, :, :, 256:264], 1.0)
        ctxT = sg.tile([128, KD, S], F8, name="ctxT")
        x2_sb = sg.tile([128, NT, D], F32, name="x2_sb")
        xn2T = sg.tile([128, KD, S], BF16, name="xn2T")
        x2bT = sg.tile([128, KD, S], BF16, name="x2bT")
        h1T = sg.tile([128, MT, S], BF16, name="h1T")

        def layernorm_to_bf16(x_ap, tag):
            """32*(x-mu)/sqrt(var+eps) -> bf16 tile."""
            st6 = p_tiny.tile([128, 2, 6], F32, tag="st6")
            nc.vector.bn_stats(st6[:, 0, :], x_ap[:, 0:384])
            nc.vector.bn_stats(st6[:, 1, :], x_ap[:, 384:768])
            mv = p_tiny.tile([128, 2], F32, tag="mv")
            nc.vector.bn_aggr(mv[:], st6[:])
            sd = p_tiny.tile([128, 1], F32, tag="sd")
            # sqrt((var+eps)/1024) = sd/32
            nc.scalar.activation(sd[:], mv[:, 1:2], AF.Sqrt, bias=eps_t[:],
                                 scale=1.0 / 1024.0)
            ri = p_tiny.tile([128, 1], F32, tag="ri")
            nc.vector.reciprocal_approx_fast(ri[:], sd[:])
            nm = p_tiny.tile([128, 1], F32, tag="nm")
            nc.vector.tensor_scalar_mul(nm[:], mv[:, 0:1], -1.0)
            xnb = p_xn.tile([128, D], BF16, tag=tag)
            nc.vector.tensor_scalar(xnb[:], x_ap, nm[:], ri[:], ALU.add,
                                    op1=ALU.mult)
            return xnb

        def transpose_to(ps_pool, xnb, dstT, i):
            pt = ps_pool.tile([128, KD, 128], BF16, tag="tp")
            for c in range(KD):
                nc.tensor.matmul(pt[:, c, :], xnb[:, ts(c, 128)], ident_bf[:],
                                 is_transpose=True, start=(c == 0),
                                 stop=(c == KD - 1), skip_group_check=True)
            nc.scalar.activation(dstT[:, :, ts(i, 128)], pt[:], AF.Copy)

        # ========= Phase A: LN1 -> xnT, V projection interleaved =========
        with ExitStack() as sa:
            ps_tp = sa.enter_context(tc.tile_pool(name="ps_tp", bufs=2, space="PSUM"))
            ps_v = sa.enter_context(tc.tile_pool(name="ps_v", bufs=2, space="PSUM"))
            for i in range(NT):
                xt = p_x.tile([128, D], F32, tag="x")
                nc.sync.dma_start(xt[:], d["x"][ts(i, 128), :])
                xnb = layernorm_to_bf16(xt[:], "xn")
                transpose_to(ps_tp, xnb, xnT, i)
                vp = ps_v.tile([128, 2, 512], F32, tag="vp")
                for k in range(3):
                    for ch in range(2):
                        nc.tensor.matmul(vp[:, ch, 0:384],
                                         xnT[:, 2 * k:2 * k + 2, ts(i, 128)],
                                         wv_sb[:, k, :, ts(ch, 384)],
                                         start=(k == 0), stop=(k == 2),
                                         perf_mode=DR)
                for ch in range(2):
                    nc.vector.tensor_scalar_mul(
                        v_aug[:, i, 6 * ch:6 * ch + 6, 0:DH],
                        vp[:, ch, 0:384].rearrange("p (h d) -> p h d", h=6),
                        1.0 / 64.0)

        # ================= Phase B: Q/K projections =================
        with ExitStack() as sb_:
            ps_q = sb_.enter_context(tc.tile_pool(name="ps_q", bufs=1, space="PSUM"))
            ps_k = sb_.enter_context(tc.tile_pool(name="ps_k", bufs=1, space="PSUM"))
            for j in range(KD):
                qp = ps_q.tile([128, SQ], F32, tag="qp")
                for k in range(3):
                    nc.tensor.matmul(qp[:], wq_sb[:, k, :, ts(j, 128)],
                                     xnT[:, 2 * k:2 * k + 2, 0:SQ],
                                     start=(k == 0), stop=(k == 2), perf_mode=DR)
                nc.vector.tensor_scalar(qt[:, j, :], qp[:], 1.0 / 64.0,
                                        bq_sb[:, j:j + 1], ALU.mult, op1=ALU.add)
                kp = ps_k.tile([128, 2, 512], F32, tag="kp")
                for k in range(3):
                    for sh in range(2):
                        nc.tensor.matmul(kp[:, sh, :], wk_sb[:, k, :, ts(j, 128)],
                                         xnT[:, 2 * k:2 * k + 2, ts(sh, 512)],
                                         start=(k == 0), stop=(k == 2),
                                         perf_mode=DR)
                for sh in range(2):
                    nc.vector.tensor_scalar(kt[:, j, ts(sh, 512)], kp[:, sh, :],
                                            1.0 / 64.0, bk_sb[:, j:j + 1],
                                            ALU.mult, op1=ALU.add)

        # ================= Phase C: scores/exp/ctx per head =================
        with ExitStack() as sc:
            ps_s = sc.enter_context(tc.tile_pool(name="ps_s", bufs=2, space="PSUM"))
            ps_c = sc.enter_context(tc.tile_pool(name="ps_c", bufs=2, space="PSUM"))
            ps_b = sc.enter_context(tc.tile_pool(name="ps_b", bufs=1, space="PSUM"))

            def ctx_head(h):
                j, r0 = h // 2, 64 * (h % 2)
                pc = [ps_c.tile([65, 512], F32, tag="pc", name=f"pc{c}")
                      for c in range(2)]
                for tp_ in range(4):
                    for c in range(2):
                        nc.tensor.matmul(
                            pc[c][:, 0:264],
                            v_aug[:, 2 * tp_:2 * tp_ + 2, h, 0:65],
                            expT[h % 2][:, 2 * tp_:2 * tp_ + 2, c, :],
                            start=(tp_ == 0), stop=(tp_ == 3), perf_mode=DR)
                for c in range(2):
                    den = p_tiny.tile([1, 256], F32R, tag="den")
                    nc.vector.tensor_copy(den[:], pc[c][DH:DH + 1, 0:256])
                    pb = ps_b.tile([64, 256], F32, tag="pb")
                    nc.tensor.matmul(pb[:], ones_r[:, 0:64], den[:],
                                     start=True, stop=True)
                    pbs = p_pbs.tile([64, 256], F32, tag="pbs")
                    nc.vector.reciprocal_approx_fast(pbs[:], pb[:])
                    nc.vector.tensor_tensor(ctxT[r0:r0 + 64, j, ts(c, 256)],
                                            pc[c][0:DH, 0:256], pbs[:], ALU.mult)
                nc.vector.tensor_scalar_mul(
                    ctxT[r0:r0 + 64, j, SQ:S],
                    pc[0][0:DH, 256:257].to_broadcast((DH, SQ)), 1.0 / 1024.0)

            pending = None
            for h in range(H):
                j, r0 = h // 2, 64 * (h % 2)
                for tp_ in range(4):
                    sp = ps_s.tile([128, 2, SQ], F32, tag="sp")
                    for u in range(2):
                        t = 2 * tp_ + u
                        nc.tensor.matmul(sp[:, u, :],
                                         kt[r0:r0 + 64, j, ts(t, 128)],
                                         qt[r0:r0 + 64, j, :],
                                         start=True, stop=True)
                    dst = expT[h % 2][:, 2 * tp_:2 * tp_ + 2, :, 0:256]
                    if h % 2 == 0:
                        nc.scalar.activation(
                            dst, sp[:].rearrange("p a (c q) -> p a c q", c=2),
                            AF.Exp, bias=negtwo[:], scale=1.0 / 8192.0)
                    else:
                        # Schraudolph: int32(sp*SA+SB) bits reread as fp32
                        nc.vector.tensor_scalar(sp[:].bitcast(I32), sp[:],
                                                SA, SB, ALU.mult, op1=ALU.add)
                        nc.vector.tensor_copy(
                            dst, sp[:].rearrange("p a (c q) -> p a c q", c=2))
                if pending is not None:
                    ctx_head(pending)
                pending = h
            ctx_head(pending)

        # ================= Phase D: attn-out + residual + LN2 =================
        with ExitStack() as sd:
            ps_a = sd.enter_context(tc.tile_pool(name="ps_a", bufs=2, space="PSUM"))
            ps_tp2 = sd.enter_context(tc.tile_pool(name="ps_tp2", bufs=2,
                                                   space="PSUM"))
            pend_tp = []
            for i in range(NT):
                pa = ps_a.tile([128, 2, 512], F32, tag="pa")
                for k in range(3):
                    for ch in range(2):
                        nc.tensor.matmul(pa[:, ch, 0:384],
                                         ctxT[:, 2 * k:2 * k + 2, ts(i, 128)],
                                         wo_sb[:, k, :, ts(ch, 384)],
                                         start=(k == 0), stop=(k == 2),
                                         perf_mode=DR)
                xre = p_x.tile([128, D], F32, tag="x")
                nc.sync.dma_start(xre[:], d["x"][ts(i, 128), :])
                xbo = p_xb.tile([128, D], BF16, tag="xbo")
                nc.gpsimd.tensor_tensor(xbo[:], xre[:], bo_bc[:], ALU.add)
                nc.scalar.activation(
                    x2_sb[:, i, :].rearrange("p (a b) -> p a b", a=2),
                    pa[:, :, 0:384], AF.Copy, scale=1.0 / 2048.0)
                nc.vector.tensor_tensor(x2_sb[:, i, :], x2_sb[:, i, :],
                                        xbo[:], ALU.add)
                xn2b = layernorm_to_bf16(x2_sb[:, i, :], "xn2")
                pend_tp.append((xn2b, i))
                if len(pend_tp) >= 2:
                    xb_, i_ = pend_tp.pop(0)
                    transpose_to(ps_tp2, xb_, xn2T, i_)
            for xb_, i_ in pend_tp:
                transpose_to(ps_tp2, xb_, xn2T, i_)

        # ================= Phase F: MLP1 + x2bT prep =================
        with ExitStack() as sf:
            ps_m = sf.enter_context(tc.tile_pool(name="ps_m", bufs=2, space="PSUM"))
            ps_tp3 = sf.enter_context(tc.tile_pool(name="ps_tp3", bufs=2,
                                                   space="PSUM"))
            for m in range(MT):
                w1t = p_w1.tile([128, KD, 128], BF16, tag="w1t")
                nc.sync.dma_start(w1t[:], d["w1b"][:, :, ts(m, 128)])
                pm = ps_m.tile([128, 2, 512], F32, tag="pm")
                for k in range(KD):
                    for sh in range(2):
                        nc.tensor.matmul(pm[:, sh, :], w1t[:, k, :],
                                         xn2T[:, k, ts(sh, 512)],
                                         start=(k == 0), stop=(k == KD - 1))
                nc.scalar.activation(
                    h1T[:, m, :].rearrange("p (a b) -> p a b", a=2), pm[:],
                    AF.Relu, bias=b1_sb[:, m:m + 1])
                if m % 3 == 2:
                    i = m // 3
                    xb2 = p_xb.tile([128, D], BF16, tag="xb2")
                    nc.vector.tensor_tensor(xb2[:], x2_sb[:, i, :], b2_bc[:],
                                            ALU.add)
                    transpose_to(ps_tp3, xb2, x2bT, i)

        # ================= Phase G: MLP2 (feature-major out) =================
        with ExitStack() as sg_:
            ps_o = sg_.enter_context(tc.tile_pool(name="ps_o", bufs=2, space="PSUM"))
            for j in range(KD):
                po = ps_o.tile([128, 2, 512], F32, tag="po")
                for half in range(2):
                    w2t = p_w2.tile([128, 12, 128], BF16, tag="w2t")
                    nc.sync.dma_start(
                        w2t[:], d["w2b"][:, 12 * half:12 * half + 12, ts(j, 128)])
                    for mi in range(12):
                        m = 12 * half + mi
                        for sh in range(2):
                            nc.tensor.matmul(po[:, sh, :], w2t[:, mi, :],
                                             h1T[:, m, ts(sh, 512)],
                                             start=(m == 0), stop=(m == MT - 1))
                nc.vector.tensor_scalar_mul(po[:], po[:], 1.0 / 32.0)
                ot = p_out.tile([128, S], BF16, tag="ot")
                nc.vector.tensor_tensor(
                    ot[:].rearrange("p (a b) -> p a b", a=2), po[:],
                    x2bT[:, j, :].rearrange("p (a b) -> p a b", a=2), ALU.add)
                nc.sync.dma_start(out_d[ts(j, 128), :], ot[:])

    nc.compile()
    return nc


_CACHE = {}


def _get_program():
    if "nc" not in _CACHE:
        _CACHE["nc"] = build_program()
    return _CACHE["nc"]


def _prep_inputs(inputs):
    import ml_dtypes

    f = lambda a: np.asarray(a, dtype=np.float32)
    x = np.ascontiguousarray(f(inputs["x"]))
    Wq, Wk, Wv, Wo = f(inputs["Wq"]), f(inputs["Wk"]), f(inputs["Wv"]), f(inputs["Wo"])
    W1, W2 = f(inputs["W1"]), f(inputs["W2"])
    bq, bk, bv, bo = f(inputs["bq"]), f(inputs["bk"]), f(inputs["bv"]), f(inputs["bo"])
    b1, b2 = f(inputs["b1"]), f(inputs["b2"])
    g1, b1l = f(inputs["ln1_g"]), f(inputs["ln1_b"])
    g2, b2l = f(inputs["ln2_g"]), f(inputs["ln2_b"])

    bq_e = bq + b1l @ Wq
    bk_e = bk + b1l @ Wk
    bv_e = bv + b1l @ Wv
    bo_e = bo + bv_e @ Wo
    b1_e = b1 + b2l @ W1

    to8 = lambda w: np.clip(w * 64.0, -240.0, 240.0).astype(ml_dtypes.float8_e4m3)

    def lay(w, kt_, m):
        return np.ascontiguousarray(
            to8(w).reshape(kt_, 2, 128, m).transpose(2, 0, 1, 3))

    tob = lambda w: w.astype(ml_dtypes.bfloat16)
    shared = {
        "wq": lay(g1[:, None] * Wq, 3, D),
        "wk": lay(g1[:, None] * Wk, 3, D),
        "wv": lay(g1[:, None] * Wv, 3, D),
        "wo": lay(Wo, 3, D),
        "w1b": np.ascontiguousarray(
            tob(g2[:, None] * W1).reshape(KD, 128, MLP).transpose(1, 0, 2)),
        "w2b": np.ascontiguousarray(
            tob(W2).reshape(MT, 128, D).transpose(1, 0, 2)),
        "bq32": 32.0 * bq_e,
        "bk32": 32.0 * bk_e,
        "b1s": 32.0 * b1_e,
        "bo_e": bo_e,
        "b2e": b2,
    }
    return x, shared


def kernel(**inputs) -> np.ndarray:
    from concourse.bass_utils import run_bass_kernel_spmd

    nc = _get_program()
    x, shared = _prep_inputs(inputs)
    in_maps = [dict(shared, x=np.ascontiguousarray(x[b])) for b in range(B)]
    res = run_bass_kernel_spmd(nc, in_maps, list(range(B)))
    return np.stack(
        [res.results[b]["out"].astype(np.float32).T for b in range(B)], axis=0
    )


# revision 40
# speedup vs baseline: 1.0841x; 1.0841x over previous
"""Trainium2 Bass kernel for a dense transformer block (pre-LN, 12-head attn + MLP).

Shapes (hardcoded): B=8, S=1024, D=768, H=12, DH=64, MLP=3072.
Query rows >= 512 have their attention scores zeroed pre-softmax, so their
context vector is mean(v) over all 1024 keys.

Sharding: data-parallel over batch - each of the 8 NeuronCores processes one
batch element; no collectives.

v2 design: fp8(e4m3) everywhere on the PE with DoubleRow (2 fp8 MACs/cell),
all weights SBUF-resident, LN gamma/beta folded into the weights on the host,
single ACT table set (ln/exp/relu/copy), feature-major MLP2 output with a
free host-side transpose.

Scale conventions (host pre-scales, on-chip rescales):
  weights fp8 = 64*W_eff, activations fp8 = 32*act. PSUM products carry
  2048x, rescaled during PSUM evacuation (1/64 into fp8 acts, 1/2048 into
  fp32). exp computed as exp(s_true - 2) for fp8 headroom (cancels in
  softmax).
"""
import sys

try:
    import concourse  # noqa: F401
except ImportError:
    sys.path.insert(0, "/opt/trn_rl_repo")

import numpy as np
from contextlib import ExitStack

import concourse.bass as bass  # noqa: F401
import concourse.tile as tile
from concourse import bacc, mybir
from concourse.bass import ts
from concourse.masks import make_identity

F32 = mybir.dt.float32
F32R = mybir.dt.float32r
F8 = mybir.dt.float8e4
BF16 = mybir.dt.bfloat16
I32 = mybir.dt.int32
AF = mybir.ActivationFunctionType
ALU = mybir.AluOpType
DR = mybir.MatmulPerfMode.DoubleRow

B, S, D = 8, 1024, 768
H, DH, MLP = 12, 64, 3072
SQ = 512          # live query rows (rows >= SQ get uniform attention)
EPS = 1e-6
KD = D // 128     # 6
NT = S // 128     # 8
MT = MLP // 128   # 24

# Schraudolph exp directly into fp8e4 bits: exp(sp/8192 - 2) ~= bitcast_e4m3(
# uint8(sp*SA8 + SB8)); uint8 saturation clamps the underflow side to +0.
SA8 = float(8.0 / (np.log(2.0) * 8192.0))
SB8 = float(56.0 - 0.375 - 16.0 / np.log(2.0))


def build_program():
    nc = bacc.Bacc(
        "TRN2", target_bir_lowering=False, debug=False, enable_asserts=False
    )
    d = {}
    d["x"] = nc.dram_tensor("x", (S, D), F32, kind="ExternalInput").ap()
    for nm in ("wq", "wk", "wv", "wo"):
        d[nm] = nc.dram_tensor(nm, (128, 3, 2, D), F8, kind="ExternalInput").ap()
    d["w1b"] = nc.dram_tensor("w1b", (128, KD, MLP), BF16, kind="ExternalInput").ap()
    d["w2b"] = nc.dram_tensor("w2b", (128, MT, D), BF16, kind="ExternalInput").ap()
    for nm, n in (("bq32", D), ("bk32", D), ("b1s", MLP)):
        d[nm] = nc.dram_tensor(nm, (n,), F32, kind="ExternalInput").ap()
    for nm in ("bo_e", "b2e"):
        d[nm] = nc.dram_tensor(nm, (D,), F32R, kind="ExternalInput").ap()
    out_d = nc.dram_tensor("out", (D, S), BF16, kind="ExternalOutput").ap()

    with tile.TileContext(nc) as tc, ExitStack() as ctx:
        sg = ctx.enter_context(tc.tile_pool(name="sg", bufs=1))
        p_x = ctx.enter_context(tc.tile_pool(name="p_x", bufs=2))
        p_xn = ctx.enter_context(tc.tile_pool(name="p_xn", bufs=2))
        p_xb = ctx.enter_context(tc.tile_pool(name="p_xb", bufs=2))
        p_tiny = ctx.enter_context(tc.tile_pool(name="p_tiny", bufs=4))
        p_pbs = ctx.enter_context(tc.tile_pool(name="p_pbs", bufs=2))
        p_out = ctx.enter_context(tc.tile_pool(name="p_out", bufs=2))

        # ---- constants ----
        ident_bf = sg.tile([128, 128], BF16, name="ident_bf")
        make_identity(nc, ident_bf[:])
        eps_t = sg.tile([128, 1], F32, name="eps_t")
        nc.vector.memset(eps_t[:], EPS / 1024.0)
        negtwo = sg.tile([128, 1], F32, name="negtwo")
        nc.vector.memset(negtwo[:], -2.0)
        ones_r = sg.tile([1, 128], F32R, name="ones_r")
        nc.vector.memset(ones_r[:].bitcast(F32), 1.0)

        # ---- weights (SBUF-resident, fp8); only wv is needed in phase A ----
        wq_sb = sg.tile([128, 3, 2, D], F8, name="wq_sb")
        wk_sb = sg.tile([128, 3, 2, D], F8, name="wk_sb")
        wv_sb = sg.tile([128, 3, 2, D], F8, name="wv_sb")
        nc.sync.dma_start(wv_sb[:], d["wv"])
        wo_sb = sg.tile([128, 3, 2, D], F8, name="wo_sb")
        p_w1 = ctx.enter_context(tc.tile_pool(name="p_w1", bufs=2))
        p_w2 = ctx.enter_context(tc.tile_pool(name="p_w2", bufs=2))

        # ---- biases ----
        bq_sb = sg.tile([128, KD], F32, name="bq_sb")
        nc.sync.dma_start(bq_sb[:], d["bq32"].rearrange("(t p) -> p t", p=128))
        bk_sb = sg.tile([128, KD], F32, name="bk_sb")
        nc.sync.dma_start(bk_sb[:], d["bk32"].rearrange("(t p) -> p t", p=128))
        b1_sb = sg.tile([128, MT], F32, name="b1_sb")
        nc.sync.dma_start(b1_sb[:], d["b1s"].rearrange("(t p) -> p t", p=128))

        with ExitStack() as sbc:
            ps_bc = sbc.enter_context(
                tc.tile_pool(name="ps_bc", bufs=2, space="PSUM"))

            def bcast_row(src_1d, name):
                row = p_tiny.tile([1, D], F32R, tag="row", name=f"row_{name}")
                nc.sync.dma_start(row[:], src_1d[None, :])
                t = sg.tile([128, D], F32, name=name)
                for half in range(2):
                    pbk = ps_bc.tile([128, 384], F32, tag="bc",
                                     name=f"bc_{name}{half}")
                    nc.tensor.matmul(pbk[:], ones_r[:], row[:, ts(half, 384)],
                                     start=True, stop=True)
                    nc.vector.tensor_copy(t[:, ts(half, 384)], pbk[:])
                return t

            bo_bc = bcast_row(d["bo_e"], "bo_bc")
            b2_bc = bcast_row(d["b2e"], "b2_bc")

        # ---- big state ----
        xnT = sg.tile([128, KD, S], F8, name="xnT")
        qt = sg.tile([128, KD, SQ], F8, name="qt")
        kt = sg.tile([128, KD, S], F8, name="kt")
        v_aug = sg.tile([128, NT, H, 68], F8, name="v_aug")
        nc.gpsimd.memset(v_aug[:, :, :, DH:DH + 1], 1.0)
        expT = [sg.tile([128, NT, 2, 264], F8, name=f"expT{i}") for i in range(2)]
        for i in range(2):
            nc.gpsimd.memset(expT[i][:, :, :, 256:264], 1.0)
        ctxT = sg.tile([128, KD, S], F8, name="ctxT")
        x2_sb = sg.tile([128, NT, D], F32, name="x2_sb")
        xn2T = sg.tile([128, KD, S], BF16, name="xn2T")
        x2bT = sg.tile([128, KD, S], BF16, name="x2bT")
        h1T = sg.tile([128, MT, S], BF16, name="h1T")

        def layernorm_to_bf16(x_ap, tag):
            """32*(x-mu)/sqrt(var+eps) -> bf16 tile."""
            st6 = p_tiny.tile([128, 2, 6], F32, tag="st6")
            nc.vector.bn_stats(st6[:, 0, :], x_ap[:, 0:384])
            nc.vector.bn_stats(st6[:, 1, :], x_ap[:, 384:768])
            mv = p_tiny.tile([128, 2], F32, tag="mv")
            nc.vector.bn_aggr(mv[:], st6[:])
            sd = p_tiny.tile([128, 1], F32, tag="sd")
            # sqrt((var+eps)/1024) = sd/32
            nc.scalar.activation(sd[:], mv[:, 1:2], AF.Sqrt, bias=eps_t[:],
                                 scale=1.0 / 1024.0)
            ri = p_tiny.tile([128, 1], F32, tag="ri")
            nc.vector.reciprocal_approx_fast(ri[:], sd[:])
            nm = p_tiny.tile([128, 1], F32, tag="nm")
            nc.vector.tensor_scalar_mul(nm[:], mv[:, 0:1], -1.0)
            xnb = p_xn.tile([128, D], BF16, tag=tag)
            nc.vector.tensor_scalar(xnb[:], x_ap, nm[:], ri[:], ALU.add,
                                    op1=ALU.mult)
            return xnb

        def transpose_to(ps_pool, xnb, dstT, i):
            pt = ps_pool.tile([128, KD, 128], BF16, tag="tp")
            for c in range(KD):
                nc.tensor.matmul(pt[:, c, :], xnb[:, ts(c, 128)], ident_bf[:],
                                 is_transpose=True, start=(c == 0),
                                 stop=(c == KD - 1), skip_group_check=True)
            nc.scalar.activation(dstT[:, :, ts(i, 128)], pt[:], AF.Copy)

        # ========= Phase A: LN1 -> xnT, V projection interleaved =========
        with ExitStack() as sa:
            ps_tp = sa.enter_context(tc.tile_pool(name="ps_tp", bufs=2, space="PSUM"))
            ps_v = sa.enter_context(tc.tile_pool(name="ps_v", bufs=2, space="PSUM"))
            for i in range(NT):
                xt = p_x.tile([128, D], F32, tag="x")
                nc.sync.dma_start(xt[:], d["x"][ts(i, 128), :])
                xnb = layernorm_to_bf16(xt[:], "xn")
                transpose_to(ps_tp, xnb, xnT, i)
                vp = ps_v.tile([128, 2, 512], F32, tag="vp")
                for k in range(3):
                    for ch in range(2):
                        nc.tensor.matmul(vp[:, ch, 0:384],
                                         xnT[:, 2 * k:2 * k + 2, ts(i, 128)],
                                         wv_sb[:, k, :, ts(ch, 384)],
                                         start=(k == 0), stop=(k == 2),
                                         perf_mode=DR)
                for ch in range(2):
                    nc.vector.tensor_scalar_mul(
                        v_aug[:, i, 6 * ch:6 * ch + 6, 0:DH],
                        vp[:, ch, 0:384].rearrange("p (h d) -> p h d", h=6),
                        1.0 / 64.0)

        # ================= Phase B: Q/K projections =================
        nc.sync.dma_start(wq_sb[:], d["wq"])
        nc.sync.dma_start(wk_sb[:], d["wk"])
        nc.sync.dma_start(wo_sb[:], d["wo"])
        with ExitStack() as sb_:
            ps_q = sb_.enter_context(tc.tile_pool(name="ps_q", bufs=1, space="PSUM"))
            ps_k = sb_.enter_context(tc.tile_pool(name="ps_k", bufs=1, space="PSUM"))
            for j in range(KD):
                qp = ps_q.tile([128, SQ], F32, tag="qp")
                for k in range(3):
                    nc.tensor.matmul(qp[:], wq_sb[:, k, :, ts(j, 128)],
                                     xnT[:, 2 * k:2 * k + 2, 0:SQ],
                                     start=(k == 0), stop=(k == 2), perf_mode=DR)
                nc.vector.tensor_scalar(qt[:, j, :], qp[:], 1.0 / 64.0,
                                        bq_sb[:, j:j + 1], ALU.mult, op1=ALU.add)
                kp = ps_k.tile([128, 2, 512], F32, tag="kp")
                for k in range(3):
                    for sh in range(2):
                        nc.tensor.matmul(kp[:, sh, :], wk_sb[:, k, :, ts(j, 128)],
                                         xnT[:, 2 * k:2 * k + 2, ts(sh, 512)],
                                         start=(k == 0), stop=(k == 2),
                                         perf_mode=DR)
                for sh in range(2):
                    nc.vector.tensor_scalar(kt[:, j, ts(sh, 512)], kp[:, sh, :],
                                            1.0 / 64.0, bk_sb[:, j:j + 1],
                                            ALU.mult, op1=ALU.add)

        # ================= Phase C: scores/exp/ctx per head =================
        with ExitStack() as sc:
            ps_s = sc.enter_context(tc.tile_pool(name="ps_s", bufs=2, space="PSUM"))
            ps_c = sc.enter_context(tc.tile_pool(name="ps_c", bufs=2, space="PSUM"))
            ps_b = sc.enter_context(tc.tile_pool(name="ps_b", bufs=1, space="PSUM"))

            def ctx_head(h):
                j, r0 = h // 2, 64 * (h % 2)
                pc = [ps_c.tile([65, 512], F32, tag="pc", name=f"pc{c}")
                      for c in range(2)]
                for tp_ in range(4):
                    for c in range(2):
                        nc.tensor.matmul(
                            pc[c][:, 0:264],
                            v_aug[:, 2 * tp_:2 * tp_ + 2, h, 0:65],
                            expT[h % 2][:, 2 * tp_:2 * tp_ + 2, c, :],
                            start=(tp_ == 0), stop=(tp_ == 3), perf_mode=DR)
                for c in range(2):
                    den = p_tiny.tile([1, 256], F32R, tag="den")
                    nc.vector.tensor_copy(den[:], pc[c][DH:DH + 1, 0:256])
                    pb = ps_b.tile([64, 256], F32, tag="pb")
                    nc.tensor.matmul(pb[:], ones_r[:, 0:64], den[:],
                                     start=True, stop=True)
                    pbs = p_pbs.tile([64, 256], F32, tag="pbs")
                    nc.vector.reciprocal_approx_fast(pbs[:], pb[:])
                    nc.vector.tensor_tensor(ctxT[r0:r0 + 64, j, ts(c, 256)],
                                            pc[c][0:DH, 0:256], pbs[:], ALU.mult)
                nc.vector.tensor_scalar_mul(
                    ctxT[r0:r0 + 64, j, SQ:S],
                    pc[0][0:DH, 256:257].to_broadcast((DH, SQ)), 1.0 / 1024.0)

            pending = None
            for h in range(H):
                j, r0 = h // 2, 64 * (h % 2)
                for tp_ in range(4):
                    sp = ps_s.tile([128, 2, SQ], F32, tag="sp")
                    for u in range(2):
                        t = 2 * tp_ + u
                        nc.tensor.matmul(sp[:, u, :],
                                         kt[r0:r0 + 64, j, ts(t, 128)],
                                         qt[r0:r0 + 64, j, :],
                                         start=True, stop=True)
                    dst = expT[h % 2][:, 2 * tp_:2 * tp_ + 2, :, 0:256]
                    if h % 3 != 2:
                        nc.scalar.activation(
                            dst, sp[:].rearrange("p a (c q) -> p a c q", c=2),
                            AF.Exp, bias=negtwo[:], scale=1.0 / 8192.0)
                    else:
                        nc.vector.tensor_scalar(
                            dst.bitcast(mybir.dt.uint8),
                            sp[:].rearrange("p a (c q) -> p a c q", c=2),
                            SA8, SB8, ALU.mult, op1=ALU.add)
                if pending is not None:
                    ctx_head(pending)
                pending = h
            ctx_head(pending)

        # ================= Phase D: attn-out + residual + LN2 =================
        with ExitStack() as sd:
            ps_a = sd.enter_context(tc.tile_pool(name="ps_a", bufs=2, space="PSUM"))
            ps_tp2 = sd.enter_context(tc.tile_pool(name="ps_tp2", bufs=2,
                                                   space="PSUM"))
            pend_tp = []
            for i in range(NT):
                pa = ps_a.tile([128, 2, 512], F32, tag="pa")
                for k in range(3):
                    for ch in range(2):
                        nc.tensor.matmul(pa[:, ch, 0:384],
                                         ctxT[:, 2 * k:2 * k + 2, ts(i, 128)],
                                         wo_sb[:, k, :, ts(ch, 384)],
                                         start=(k == 0), stop=(k == 2),
                                         perf_mode=DR)
                xre = p_x.tile([128, D], F32, tag="x")
                nc.sync.dma_start(xre[:], d["x"][ts(i, 128), :])
                xbo = p_xb.tile([128, D], BF16, tag="xbo")
                nc.vector.tensor_tensor(xbo[:], xre[:], bo_bc[:], ALU.add)
                nc.scalar.activation(
                    x2_sb[:, i, :].rearrange("p (a b) -> p a b", a=2),
                    pa[:, :, 0:384], AF.Copy, scale=1.0 / 2048.0)
                nc.vector.tensor_tensor(x2_sb[:, i, :], x2_sb[:, i, :],
                                        xbo[:], ALU.add)
                xn2b = layernorm_to_bf16(x2_sb[:, i, :], "xn2")
                pend_tp.append((xn2b, i))
                if len(pend_tp) >= 2:
                    xb_, i_ = pend_tp.pop(0)
                    transpose_to(ps_tp2, xb_, xn2T, i_)
            for xb_, i_ in pend_tp:
                transpose_to(ps_tp2, xb_, xn2T, i_)

        # ================= Phase F: MLP1 + x2bT prep =================
        with ExitStack() as sf:
            ps_m = sf.enter_context(tc.tile_pool(name="ps_m", bufs=2, space="PSUM"))
            ps_tp3 = sf.enter_context(tc.tile_pool(name="ps_tp3", bufs=2,
                                                   space="PSUM"))
            for m in range(MT):
                w1t = p_w1.tile([128, KD, 128], BF16, tag="w1t")
                nc.sync.dma_start(w1t[:], d["w1b"][:, :, ts(m, 128)])
                pm = ps_m.tile([128, 2, 512], F32, tag="pm")
                for k in range(KD):
                    for sh in range(2):
                        nc.tensor.matmul(pm[:, sh, :], w1t[:, k, :],
                                         xn2T[:, k, ts(sh, 512)],
                                         start=(k == 0), stop=(k == KD - 1))
                nc.scalar.activation(
                    h1T[:, m, :].rearrange("p (a b) -> p a b", a=2), pm[:],
                    AF.Relu, bias=b1_sb[:, m:m + 1])
                if m % 3 == 2:
                    i = m // 3
                    xb2 = p_xb.tile([128, D], BF16, tag="xb2")
                    nc.vector.tensor_tensor(xb2[:], x2_sb[:, i, :], b2_bc[:],
                                            ALU.add)
                    transpose_to(ps_tp3, xb2, x2bT, i)

        # ================= Phase G: MLP2 (feature-major out) =================
        with ExitStack() as sg_:
            ps_o = sg_.enter_context(tc.tile_pool(name="ps_o", bufs=2, space="PSUM"))
            for j in range(KD):
                po = ps_o.tile([128, 2, 512], F32, tag="po")
                for half in range(2):
                    w2t = p_w2.tile([128, 12, 128], BF16, tag="w2t")
                    nc.sync.dma_start(
                        w2t[:], d["w2b"][:, 12 * half:12 * half + 12, ts(j, 128)])
                    for mi in range(12):
                        m = 12 * half + mi
                        for sh in range(2):
                            nc.tensor.matmul(po[:, sh, :], w2t[:, mi, :],
                                             h1T[:, m, ts(sh, 512)],
                                             start=(m == 0), stop=(m == MT - 1))
                nc.vector.tensor_scalar_mul(po[:], po[:], 1.0 / 32.0)
                ot = p_out.tile([128, S], BF16, tag="ot")
                nc.vector.tensor_tensor(
                    ot[:].rearrange("p (a b) -> p a b", a=2), po[:],
                    x2bT[:, j, :].rearrange("p (a b) -> p a b", a=2), ALU.add)
                nc.sync.dma_start(out_d[ts(j, 128), :], ot[:])

    nc.compile()
    return nc


_CACHE = {}


def _get_program():
    if "nc" not in _CACHE:
        _CACHE["nc"] = build_program()
    return _CACHE["nc"]


def _prep_inputs(inputs):
    import ml_dtypes

    f = lambda a: np.asarray(a, dtype=np.float32)
    x = np.ascontiguousarray(f(inputs["x"]))
    Wq, Wk, Wv, Wo = f(inputs["Wq"]), f(inputs["Wk"]), f(inputs["Wv"]), f(inputs["Wo"])
    W1, W2 = f(inputs["W1"]), f(inputs["W2"])
    bq, bk, bv, bo = f(inputs["bq"]), f(inputs["bk"]), f(inputs["bv"]), f(inputs["bo"])
    b1, b2 = f(inputs["b1"]), f(inputs["b2"])
    g1, b1l = f(inputs["ln1_g"]), f(inputs["ln1_b"])
    g2, b2l = f(inputs["ln2_g"]), f(inputs["ln2_b"])

    bq_e = bq + b1l @ Wq
    bk_e = bk + b1l @ Wk
    bv_e = bv + b1l @ Wv
    bo_e = bo + bv_e @ Wo
    b1_e = b1 + b2l @ W1

    to8 = lambda w: np.clip(w * 64.0, -240.0, 240.0).astype(ml_dtypes.float8_e4m3)

    def lay(w, kt_, m):
        return np.ascontiguousarray(
            to8(w).reshape(kt_, 2, 128, m).transpose(2, 0, 1, 3))

    tob = lambda w: w.astype(ml_dtypes.bfloat16)
    shared = {
        "wq": lay(g1[:, None] * Wq, 3, D),
        "wk": lay(g1[:, None] * Wk, 3, D),
        "wv": lay(g1[:, None] * Wv, 3, D),
        "wo": lay(Wo, 3, D),
        "w1b": np.ascontiguousarray(
            tob(g2[:, None] * W1).reshape(KD, 128, MLP).transpose(1, 0, 2)),
        "w2b": np.ascontiguousarray(
            tob(W2).reshape(MT, 128, D).transpose(1, 0, 2)),
        "bq32": 32.0 * bq_e,
        "bk32": 32.0 * bk_e,
        "b1s": 32.0 * b1_e,
        "bo_e": bo_e,
        "b2e": b2,
    }
    return x, shared


def kernel(**inputs) -> np.ndarray:
    from concourse.bass_utils import run_bass_kernel_spmd

    nc = _get_program()
    x, shared = _prep_inputs(inputs)
    in_maps = [dict(shared, x=np.ascontiguousarray(x[b])) for b in range(B)]
    res = run_bass_kernel_spmd(nc, in_maps, list(range(B)))
    return np.stack(
        [res.results[b]["out"].astype(np.float32).T for b in range(B)], axis=0
    )


# revision 42
# speedup vs baseline: 1.1055x; 1.0198x over previous
"""Trainium2 Bass kernel for a dense transformer block (pre-LN, 12-head attn + MLP).

Shapes (hardcoded): B=8, S=1024, D=768, H=12, DH=64, MLP=3072.
Query rows >= 512 have their attention scores zeroed pre-softmax, so their
context vector is mean(v) over all 1024 keys.

Sharding: data-parallel over batch - each of the 8 NeuronCores processes one
batch element; no collectives.

v2 design: fp8(e4m3) everywhere on the PE with DoubleRow (2 fp8 MACs/cell),
all weights SBUF-resident, LN gamma/beta folded into the weights on the host,
single ACT table set (ln/exp/relu/copy), feature-major MLP2 output with a
free host-side transpose.

Scale conventions (host pre-scales, on-chip rescales):
  weights fp8 = 64*W_eff, activations fp8 = 32*act. PSUM products carry
  2048x, rescaled during PSUM evacuation (1/64 into fp8 acts, 1/2048 into
  fp32). exp computed as exp(s_true - 2) for fp8 headroom (cancels in
  softmax).
"""
import sys

try:
    import concourse  # noqa: F401
except ImportError:
    sys.path.insert(0, "/opt/trn_rl_repo")

import numpy as np
from contextlib import ExitStack

import concourse.bass as bass  # noqa: F401
import concourse.tile as tile
from concourse import bacc, mybir
from concourse.bass import ts
from concourse.masks import make_identity

F32 = mybir.dt.float32
F32R = mybir.dt.float32r
F8 = mybir.dt.float8e4
BF16 = mybir.dt.bfloat16
I32 = mybir.dt.int32
AF = mybir.ActivationFunctionType
ALU = mybir.AluOpType
DR = mybir.MatmulPerfMode.DoubleRow

B, S, D = 8, 1024, 768
H, DH, MLP = 12, 64, 3072
SQ = 512          # live query rows (rows >= SQ get uniform attention)
EPS = 1e-6
KD = D // 128     # 6
NT = S // 128     # 8
MT = MLP // 128   # 24

# Schraudolph exp directly into fp8e4 bits: exp(sp/8192 - 2) ~= bitcast_e4m3(
# uint8(sp*SA8 + SB8)); uint8 saturation clamps the underflow side to +0.
SA8 = float(8.0 / (np.log(2.0) * 8192.0))
SB8 = float(56.0 - 0.375 - 16.0 / np.log(2.0))


def build_program():
    nc = bacc.Bacc(
        "TRN2", target_bir_lowering=False, debug=False, enable_asserts=False
    )
    d = {}
    d["x"] = nc.dram_tensor("x", (S, D), F32, kind="ExternalInput").ap()
    for nm in ("wq", "wk", "wv", "wo"):
        d[nm] = nc.dram_tensor(nm, (128, 3, 2, D), F8, kind="ExternalInput").ap()
    d["w1b"] = nc.dram_tensor("w1b", (128, KD, MLP), BF16, kind="ExternalInput").ap()
    d["w2b"] = nc.dram_tensor("w2b", (128, MT, D), BF16, kind="ExternalInput").ap()
    for nm, n in (("bq32", D), ("bk32", D), ("b1s", MLP)):
        d[nm] = nc.dram_tensor(nm, (n,), F32, kind="ExternalInput").ap()
    for nm in ("bo_e", "b2e"):
        d[nm] = nc.dram_tensor(nm, (D,), F32R, kind="ExternalInput").ap()
    out_d = nc.dram_tensor("out", (D, S), BF16, kind="ExternalOutput").ap()

    with tile.TileContext(nc) as tc, ExitStack() as ctx:
        sg = ctx.enter_context(tc.tile_pool(name="sg", bufs=1))
        p_x = ctx.enter_context(tc.tile_pool(name="p_x", bufs=2))
        p_xn = ctx.enter_context(tc.tile_pool(name="p_xn", bufs=2))
        p_xb = ctx.enter_context(tc.tile_pool(name="p_xb", bufs=2))
        p_tiny = ctx.enter_context(tc.tile_pool(name="p_tiny", bufs=4))
        p_pbs = ctx.enter_context(tc.tile_pool(name="p_pbs", bufs=2))
        p_out = ctx.enter_context(tc.tile_pool(name="p_out", bufs=2))

        # ---- constants ----
        ident_bf = sg.tile([128, 128], BF16, name="ident_bf")
        make_identity(nc, ident_bf[:])
        eps_t = sg.tile([128, 1], F32, name="eps_t")
        nc.vector.memset(eps_t[:], EPS / 1024.0)
        negtwo = sg.tile([128, 1], F32, name="negtwo")
        nc.vector.memset(negtwo[:], -2.0)
        ones_r = sg.tile([1, 128], F32R, name="ones_r")
        nc.vector.memset(ones_r[:].bitcast(F32), 1.0)

        # ---- weights (SBUF-resident, fp8); only wv is needed in phase A ----
        wq_sb = sg.tile([128, 3, 2, D], F8, name="wq_sb")
        wk_sb = sg.tile([128, 3, 2, D], F8, name="wk_sb")
        wv_sb = sg.tile([128, 3, 2, D], F8, name="wv_sb")
        nc.sync.dma_start(wv_sb[:], d["wv"])
        wo_sb = sg.tile([128, 3, 2, D], F8, name="wo_sb")
        p_w1 = ctx.enter_context(tc.tile_pool(name="p_w1", bufs=2))
        p_w2 = ctx.enter_context(tc.tile_pool(name="p_w2", bufs=2))

        # ---- biases ----
        bq_sb = sg.tile([128, KD], F32, name="bq_sb")
        nc.sync.dma_start(bq_sb[:], d["bq32"].rearrange("(t p) -> p t", p=128))
        bk_sb = sg.tile([128, KD], F32, name="bk_sb")
        nc.sync.dma_start(bk_sb[:], d["bk32"].rearrange("(t p) -> p t", p=128))
        b1_sb = sg.tile([128, MT], F32, name="b1_sb")
        nc.sync.dma_start(b1_sb[:], d["b1s"].rearrange("(t p) -> p t", p=128))

        with ExitStack() as sbc:
            ps_bc = sbc.enter_context(
                tc.tile_pool(name="ps_bc", bufs=2, space="PSUM"))

            def bcast_row(src_1d, name):
                row = p_tiny.tile([1, D], F32R, tag="row", name=f"row_{name}")
                nc.sync.dma_start(row[:], src_1d[None, :])
                t = sg.tile([128, D], F32, name=name)
                for half in range(2):
                    pbk = ps_bc.tile([128, 384], F32, tag="bc",
                                     name=f"bc_{name}{half}")
                    nc.tensor.matmul(pbk[:], ones_r[:], row[:, ts(half, 384)],
                                     start=True, stop=True)
                    nc.vector.tensor_copy(t[:, ts(half, 384)], pbk[:])
                return t

            bo_bc = bcast_row(d["bo_e"], "bo_bc")
            b2_bc = bcast_row(d["b2e"], "b2_bc")

        # ---- big state ----
        xnT = sg.tile([128, KD, S], F8, name="xnT")
        qt = sg.tile([128, KD, SQ], F8, name="qt")
        kt = sg.tile([128, KD, S], F8, name="kt")
        v_aug = sg.tile([128, NT, H, 68], F8, name="v_aug")
        nc.gpsimd.memset(v_aug[:, :, :, DH:DH + 1], 1.0)
        expT = [sg.tile([128, NT, 2, 264], F8, name=f"expT{i}") for i in range(2)]
        for i in range(2):
            nc.gpsimd.memset(expT[i][:, :, :, 256:264], 1.0)
        ctxT = sg.tile([128, KD, S], F8, name="ctxT")
        x2_sb = sg.tile([128, NT, D], F32, name="x2_sb")
        xn2T = sg.tile([128, KD, S], BF16, name="xn2T")
        x2bT = sg.tile([128, KD, S], BF16, name="x2bT")
        h1T = sg.tile([128, MT, S], BF16, name="h1T")

        def layernorm_to_bf16(x_ap, tag):
            """32*(x-mu)/sqrt(var+eps) -> bf16 tile."""
            st6 = p_tiny.tile([128, 2, 6], F32, tag="st6")
            nc.vector.bn_stats(st6[:, 0, :], x_ap[:, 0:384])
            nc.vector.bn_stats(st6[:, 1, :], x_ap[:, 384:768])
            mv = p_tiny.tile([128, 2], F32, tag="mv")
            nc.vector.bn_aggr(mv[:], st6[:])
            sd = p_tiny.tile([128, 1], F32, tag="sd")
            # sqrt((var+eps)/1024) = sd/32
            nc.scalar.activation(sd[:], mv[:, 1:2], AF.Sqrt, bias=eps_t[:],
                                 scale=1.0 / 1024.0)
            ri = p_tiny.tile([128, 1], F32, tag="ri")
            nc.vector.reciprocal_approx_fast(ri[:], sd[:])
            nm = p_tiny.tile([128, 1], F32, tag="nm")
            nc.vector.tensor_scalar_mul(nm[:], mv[:, 0:1], -1.0)
            xnb = p_xn.tile([128, D], BF16, tag=tag)
            nc.vector.tensor_scalar(xnb[:], x_ap, nm[:], ri[:], ALU.add,
                                    op1=ALU.mult)
            return xnb

        def transpose_to(ps_pool, xnb, dstT, i):
            pt = ps_pool.tile([128, KD, 128], BF16, tag="tp")
            for c in range(KD):
                nc.tensor.matmul(pt[:, c, :], xnb[:, ts(c, 128)], ident_bf[:],
                                 is_transpose=True, start=(c == 0),
                                 stop=(c == KD - 1), skip_group_check=True)
            nc.scalar.activation(dstT[:, :, ts(i, 128)], pt[:], AF.Copy)

        # ========= Phase A: LN1 -> xnT, V projection interleaved =========
        with ExitStack() as sa:
            ps_tp = sa.enter_context(tc.tile_pool(name="ps_tp", bufs=2, space="PSUM"))
            ps_v = sa.enter_context(tc.tile_pool(name="ps_v", bufs=2, space="PSUM"))
            for i in range(NT):
                xt = p_x.tile([128, D], F32, tag="x")
                nc.sync.dma_start(xt[:], d["x"][ts(i, 128), :])
                xnb = layernorm_to_bf16(xt[:], "xn")
                transpose_to(ps_tp, xnb, xnT, i)
                vp = ps_v.tile([128, 2, 512], F32, tag="vp")
                for k in range(3):
                    for ch in range(2):
                        nc.tensor.matmul(vp[:, ch, 0:384],
                                         xnT[:, 2 * k:2 * k + 2, ts(i, 128)],
                                         wv_sb[:, k, :, ts(ch, 384)],
                                         start=(k == 0), stop=(k == 2),
                                         perf_mode=DR)
                for ch in range(2):
                    nc.vector.tensor_scalar_mul(
                        v_aug[:, i, 6 * ch:6 * ch + 6, 0:DH],
                        vp[:, ch, 0:384].rearrange("p (h d) -> p h d", h=6),
                        1.0 / 64.0)

        # ================= Phase B: Q/K projections =================
        nc.sync.dma_start(wq_sb[:], d["wq"])
        nc.sync.dma_start(wk_sb[:], d["wk"])
        nc.sync.dma_start(wo_sb[:], d["wo"])
        with ExitStack() as sb_:
            ps_q = sb_.enter_context(tc.tile_pool(name="ps_q", bufs=2, space="PSUM"))
            ps_k = sb_.enter_context(tc.tile_pool(name="ps_k", bufs=2, space="PSUM"))
            for j in range(KD):
                qp = ps_q.tile([128, SQ], F32, tag="qp")
                for k in range(3):
                    nc.tensor.matmul(qp[:], wq_sb[:, k, :, ts(j, 128)],
                                     xnT[:, 2 * k:2 * k + 2, 0:SQ],
                                     start=(k == 0), stop=(k == 2), perf_mode=DR)
                nc.vector.tensor_scalar(qt[:, j, :], qp[:], 1.0 / 64.0,
                                        bq_sb[:, j:j + 1], ALU.mult, op1=ALU.add)
                kp = ps_k.tile([128, 2, 512], F32, tag="kp")
                for k in range(3):
                    for sh in range(2):
                        nc.tensor.matmul(kp[:, sh, :], wk_sb[:, k, :, ts(j, 128)],
                                         xnT[:, 2 * k:2 * k + 2, ts(sh, 512)],
                                         start=(k == 0), stop=(k == 2),
                                         perf_mode=DR)
                for sh in range(2):
                    nc.vector.tensor_scalar(kt[:, j, ts(sh, 512)], kp[:, sh, :],
                                            1.0 / 64.0, bk_sb[:, j:j + 1],
                                            ALU.mult, op1=ALU.add)

        # ================= Phase C: scores/exp/ctx per head =================
        with ExitStack() as sc:
            ps_s = sc.enter_context(tc.tile_pool(name="ps_s", bufs=2, space="PSUM"))
            ps_c = sc.enter_context(tc.tile_pool(name="ps_c", bufs=2, space="PSUM"))
            ps_b = sc.enter_context(tc.tile_pool(name="ps_b", bufs=1, space="PSUM"))

            def ctx_head(h):
                j, r0 = h // 2, 64 * (h % 2)
                pc = [ps_c.tile([65, 512], F32, tag="pc", name=f"pc{c}")
                      for c in range(2)]
                for tp_ in range(4):
                    for c in range(2):
                        nc.tensor.matmul(
                            pc[c][:, 0:264],
                            v_aug[:, 2 * tp_:2 * tp_ + 2, h, 0:65],
                            expT[h % 2][:, 2 * tp_:2 * tp_ + 2, c, :],
                            start=(tp_ == 0), stop=(tp_ == 3), perf_mode=DR)
                for c in range(2):
                    den = p_tiny.tile([1, 256], F32R, tag="den")
                    nc.vector.tensor_copy(den[:], pc[c][DH:DH + 1, 0:256])
                    pb = ps_b.tile([64, 256], F32, tag="pb")
                    nc.tensor.matmul(pb[:], ones_r[:, 0:64], den[:],
                                     start=True, stop=True)
                    pbs = p_pbs.tile([64, 256], F32, tag="pbs")
                    nc.vector.reciprocal_approx_fast(pbs[:], pb[:])
                    nc.vector.tensor_tensor(ctxT[r0:r0 + 64, j, ts(c, 256)],
                                            pc[c][0:DH, 0:256], pbs[:], ALU.mult)
                nc.vector.tensor_scalar_mul(
                    ctxT[r0:r0 + 64, j, SQ:S],
                    pc[0][0:DH, 256:257].to_broadcast((DH, SQ)), 1.0 / 1024.0)

            pending = None
            for h in range(H):
                j, r0 = h // 2, 64 * (h % 2)
                for tp_ in range(4):
                    sp = ps_s.tile([128, 2, SQ], F32, tag="sp")
                    for u in range(2):
                        t = 2 * tp_ + u
                        nc.tensor.matmul(sp[:, u, :],
                                         kt[r0:r0 + 64, j, ts(t, 128)],
                                         qt[r0:r0 + 64, j, :],
                                         start=True, stop=True)
                    dst = expT[h % 2][:, 2 * tp_:2 * tp_ + 2, :, 0:256]
                    if h % 3 != 2:
                        nc.scalar.activation(
                            dst, sp[:].rearrange("p a (c q) -> p a c q", c=2),
                            AF.Exp, bias=negtwo[:], scale=1.0 / 8192.0)
                    else:
                        nc.vector.tensor_scalar(
                            dst.bitcast(mybir.dt.uint8),
                            sp[:].rearrange("p a (c q) -> p a c q", c=2),
                            SA8, SB8, ALU.mult, op1=ALU.add)
                if pending is not None:
                    ctx_head(pending)
                pending = h
            ctx_head(pending)

        # ================= Phase D: attn-out + residual + LN2 =================
        with ExitStack() as sd:
            ps_a = sd.enter_context(tc.tile_pool(name="ps_a", bufs=3, space="PSUM"))
            ps_tp2 = sd.enter_context(tc.tile_pool(name="ps_tp2", bufs=2,
                                                   space="PSUM"))
            pend_tp = []
            for i in range(NT):
                pa = ps_a.tile([128, 2, 512], F32, tag="pa")
                for k in range(3):
                    for ch in range(2):
                        nc.tensor.matmul(pa[:, ch, 0:384],
                                         ctxT[:, 2 * k:2 * k + 2, ts(i, 128)],
                                         wo_sb[:, k, :, ts(ch, 384)],
                                         start=(k == 0), stop=(k == 2),
                                         perf_mode=DR)
                xre = p_x.tile([128, D], F32, tag="x")
                nc.sync.dma_start(xre[:], d["x"][ts(i, 128), :])
                xbo = p_xb.tile([128, D], BF16, tag="xbo")
                nc.vector.tensor_tensor(xbo[:], xre[:], bo_bc[:], ALU.add)
                nc.scalar.activation(
                    x2_sb[:, i, :].rearrange("p (a b) -> p a b", a=2),
                    pa[:, :, 0:384], AF.Copy, scale=1.0 / 2048.0)
                nc.vector.tensor_tensor(x2_sb[:, i, :], x2_sb[:, i, :],
                                        xbo[:], ALU.add)
                xn2b = layernorm_to_bf16(x2_sb[:, i, :], "xn2")
                pend_tp.append((xn2b, i))
                if len(pend_tp) >= 2:
                    xb_, i_ = pend_tp.pop(0)
                    transpose_to(ps_tp2, xb_, xn2T, i_)
            for xb_, i_ in pend_tp:
                transpose_to(ps_tp2, xb_, xn2T, i_)

        # ================= Phase F: MLP1 + x2bT prep =================
        with ExitStack() as sf:
            ps_m = sf.enter_context(tc.tile_pool(name="ps_m", bufs=2, space="PSUM"))
            ps_tp3 = sf.enter_context(tc.tile_pool(name="ps_tp3", bufs=2,
                                                   space="PSUM"))
            for m in range(MT):
                w1t = p_w1.tile([128, KD, 128], BF16, tag="w1t")
                nc.sync.dma_start(w1t[:], d["w1b"][:, :, ts(m, 128)])
                pm = ps_m.tile([128, 2, 512], F32, tag="pm")
                for k in range(KD):
                    for sh in range(2):
                        nc.tensor.matmul(pm[:, sh, :], w1t[:, k, :],
                                         xn2T[:, k, ts(sh, 512)],
                                         start=(k == 0), stop=(k == KD - 1))
                nc.scalar.activation(
                    h1T[:, m, :].rearrange("p (a b) -> p a b", a=2), pm[:],
                    AF.Relu, bias=b1_sb[:, m:m + 1])
                if m % 3 == 2:
                    i = m // 3
                    xb2 = p_xb.tile([128, D], BF16, tag="xb2")
                    nc.vector.tensor_tensor(xb2[:], x2_sb[:, i, :], b2_bc[:],
                                            ALU.add)
                    transpose_to(ps_tp3, xb2, x2bT, i)

        # ================= Phase G: MLP2 (feature-major out) =================
        with ExitStack() as sg_:
            ps_o = sg_.enter_context(tc.tile_pool(name="ps_o", bufs=2, space="PSUM"))
            for j in range(KD):
                po = ps_o.tile([128, 2, 512], F32, tag="po")
                for half in range(2):
                    w2t = p_w2.tile([128, 12, 128], BF16, tag="w2t")
                    nc.sync.dma_start(
                        w2t[:], d["w2b"][:, 12 * half:12 * half + 12, ts(j, 128)])
                    for mi in range(12):
                        m = 12 * half + mi
                        for sh in range(2):
                            nc.tensor.matmul(po[:, sh, :], w2t[:, mi, :],
                                             h1T[:, m, ts(sh, 512)],
                                             start=(m == 0), stop=(m == MT - 1))
                nc.vector.tensor_scalar_mul(po[:], po[:], 1.0 / 32.0)
                ot = p_out.tile([128, S], BF16, tag="ot")
                nc.vector.tensor_tensor(
                    ot[:].rearrange("p (a b) -> p a b", a=2), po[:],
                    x2bT[:, j, :].rearrange("p (a b) -> p a b", a=2), ALU.add)
                nc.sync.dma_start(out_d[ts(j, 128), :], ot[:])

    nc.compile()
    return nc


_CACHE = {}


def _get_program():
    if "nc" not in _CACHE:
        _CACHE["nc"] = build_program()
    return _CACHE["nc"]


def _prep_inputs(inputs):
    import ml_dtypes

    f = lambda a: np.asarray(a, dtype=np.float32)
    x = np.ascontiguousarray(f(inputs["x"]))
    Wq, Wk, Wv, Wo = f(inputs["Wq"]), f(inputs["Wk"]), f(inputs["Wv"]), f(inputs["Wo"])
    W1, W2 = f(inputs["W1"]), f(inputs["W2"])
    bq, bk, bv, bo = f(inputs["bq"]), f(inputs["bk"]), f(inputs["bv"]), f(inputs["bo"])
    b1, b2 = f(inputs["b1"]), f(inputs["b2"])
    g1, b1l = f(inputs["ln1_g"]), f(inputs["ln1_b"])
    g2, b2l = f(inputs["ln2_g"]), f(inputs["ln2_b"])

    bq_e = bq + b1l @ Wq
    bk_e = bk + b1l @ Wk
    bv_e = bv + b1l @ Wv
    bo_e = bo + bv_e @ Wo
    b1_e = b1 + b2l @ W1

    to8 = lambda w: np.clip(w * 64.0, -240.0, 240.0).astype(ml_dtypes.float8_e4m3)

    def lay(w, kt_, m):
        return np.ascontiguousarray(
            to8(w).reshape(kt_, 2, 128, m).transpose(2, 0, 1, 3))

    tob = lambda w: w.astype(ml_dtypes.bfloat16)
    shared = {
        "wq": lay(g1[:, None] * Wq, 3, D),
        "wk": lay(g1[:, None] * Wk, 3, D),
        "wv": lay(g1[:, None] * Wv, 3, D),
        "wo": lay(Wo, 3, D),
        "w1b": np.ascontiguousarray(
            tob(g2[:, None] * W1).reshape(KD, 128, MLP).transpose(1, 0, 2)),
        "w2b": np.ascontiguousarray(
            tob(W2).reshape(MT, 128, D).transpose(1, 0, 2)),
        "bq32": 32.0 * bq_e,
        "bk32": 32.0 * bk_e,
        "b1s": 32.0 * b1_e,
        "bo_e": bo_e,
        "b2e": b2,
    }
    return x, shared


def kernel(**inputs) -> np.ndarray:
    from concourse.bass_utils import run_bass_kernel_spmd

    nc = _get_program()
    x, shared = _prep_inputs(inputs)
    in_maps = [dict(shared, x=np.ascontiguousarray(x[b])) for b in range(B)]
    res = run_bass_kernel_spmd(nc, in_maps, list(range(B)))
    return np.stack(
        [res.results[b]["out"].astype(np.float32).T for b in range(B)], axis=0
    )
